# revision 52
# baseline (speedup 1.0000x reference)
"""GQA self-attention kernel for Trainium2, sharded over 8 NeuronCores.

Problem: x[4, 2048, 1024], 16 heads / 4 KV groups / head_dim 64.
Sharding: batch (4) x head-half (2 KV groups each) = 8 cores.

Final dataflow (all-bf16 PE path, transposed world: features on partitions):
  xT[1024,2048]b16 -> qT[512,2048], kT[128,2048], vT[128,2048]  (PE, bf16)
  vT --PE transpose--> vaug[seq,65] tiles (ones col appended -> softmax sums)
  scores s[k,q] = kT_g^T(d,kpos) . qT_h(d,q): K=64 contraction, the two KV
    groups run CONCURRENTLY in the upper/lower 64-row halves of the PE array
    (row tiling via base_partition-derived tile_position)
  exp: ONE whole [128,1024] instruction per kt tile, alternating engines —
    even kts on ACT (exact Exp), odd kts on DVE (Schraudolph fast-exp
    emitting bf16 bits). The single wide instruction amortizes each
    engine's ~230-cycle fixed cost, and alternation keeps both exp streams
    at ~550-600ns/kt, safely under the PE's ~870ns/kt.
  av[65,q] += vaug^T p (row 64 = softmax denominator); AV lags the score/exp
    stream by 2-3 kt tiles (two kts of scores, then a 4-matmul AV block —
    grouping same-tiling-config matmuls halves the ~100ns weight-switch
    transition cost) so exp latency is fully hidden.
  pair end: av snapshot PSUM->SBUF on ACT (frees the PSUM accumulator banks
    immediately); normalize avT = av[0:64] * recip(av[64]) is deferred into
    the next pair's exp shadow (denom row replicated via K=1 f32r MM, DVE
    reciprocal, multiply on the otherwise-idle GPSIMD engine).
  yT[e,q] = Wo_p^T . avT_norm -> DRAM (bf16; host upcasts + sums partials)
The attention loop is software-pipelined; Q-projection (next chunk) and
out-projection (previous chunk) matmuls are spliced into the PE queue as
fillers so the PE stays dense (HAM stays warm).

Bias handling (exact): bk cancels in softmax (per-query constant shift);
bv is folded into the host-side output bias (bo_eff = bo + bv_exp @ Wo);
bq is applied on-device as before.

Startup: all inputs are host-prepacked into their SBUF layouts so each
tensor loads with ONE dma_start (the sync-queue DIRECT2D issue path costs
~650ns per dma_start; the v2 kernel's 97 input issues serialized ~60us).
All chunk-striped SBUF tensors (xT, kT, vT, vaug) are per-chunk tiles so
whole-tile dependency tracking never gates chunk-0 compute on chunk-3 DMAs.
Host: y[b] = (yT[2b] + yT[2b+1]).T + bo_eff
"""

import os
import sys
import numpy as np

# Coarse whole-tile dependency tracking: subtile AP-overlap analysis has
# proven racy for this kernel's 3D-sliced persistent tiles (intermittent
# first-run corruption); whole-tile deps are conservative and safe.
os.environ["BY_DEFAULT_DISABLE_SUBTILE_DEPS"] = "1"

sys.path.insert(0, "/opt/trn_rl_repo")

from collections import deque
from contextlib import ExitStack

import ml_dtypes

import concourse.bass as bass
import concourse.bacc as bacc
import concourse.mybir as mybir
from concourse import tile
from concourse.bass_utils import run_bass_kernel_spmd

F32 = mybir.dt.float32
BF16 = mybir.dt.bfloat16
NPBF16 = ml_dtypes.bfloat16

B, S, E = 4, 2048, 1024
NUM_HEADS, NUM_GROUPS, D = 16, 4, 64
CQ = 512          # q cols per core (8 heads)
CK = 128          # kv cols per core (2 groups)
ET = E // 128     # 8 embed K-tiles
SC = S // 512     # 4 seq chunks of 512
KT = S // 128     # 16 key tiles of 128
KTC = KT // SC    # 4 key tiles per chunk
QT = CQ // 128    # 4 qT partition tiles (pair p: head p @0-63, head p+4 @64-127)
SCALE = 1.0 / np.sqrt(np.float32(D))
# Schraudolph fast-exp constants emitting bf16 bits as int16:
#   bf16_bits(e^(x*SCALE)) ~= int16((2^23/ln2)*SCALE/2^16 * x + (127*2^23-c)/2^16)
# c = 486411 minimizes rms relative error (~1.8%); odd kt tiles use it on
# DVE while even kts use exact Exp on ACT — 50% of p-elements through the
# approximation gives sim/HW rel err 9.7e-3 (tolerance 2e-2).
SCHR_A = float((2.0 ** 23 / np.log(2.0)) * SCALE / 65536.0)
SCHR_B = float((127 * 2 ** 23 - 486411) / 65536.0)

_NC_CACHE = {}


def build_nc():
    nc = bacc.Bacc(None, target_bir_lowering=False)

    # All tensors host-prepacked to SBUF layout: one dma_start each.
    xTd = nc.dram_tensor("xTd", [128, ET, S], BF16, kind="ExternalInput")
    wqd = nc.dram_tensor("wqd", [128, ET, CQ], BF16, kind="ExternalInput")
    wkd = nc.dram_tensor("wkd", [128, ET, CK], BF16, kind="ExternalInput")
    wvd = nc.dram_tensor("wvd", [128, ET, CK], BF16, kind="ExternalInput")
    wod = nc.dram_tensor("wod", [128, QT, E], BF16, kind="ExternalInput")
    bqd = nc.dram_tensor("bqd", [128, QT], F32, kind="ExternalInput")
    identd = nc.dram_tensor("identd", [128, 128], BF16, kind="ExternalInput")
    onesd = nc.dram_tensor("onesd", [128, 64], BF16, kind="ExternalInput")
    onesf = nc.dram_tensor("onesf", [1, 64], mybir.dt.float32r, kind="ExternalInput")
    # bf16 output halves the 8MB/core store traffic; the host upcasts and
    # sums the two head-half partials in f32 (adds ~0.1% quantization noise,
    # well within the 2e-2 budget).
    yT = nc.dram_tensor("yT", [E, S], BF16, kind="ExternalOutput")

    ADD = mybir.AluOpType.add
    MUL = mybir.AluOpType.mult
    EXP = mybir.ActivationFunctionType.Exp

    with tile.TileContext(nc) as tc, ExitStack() as ctx, \
            nc.allow_low_precision(reason="bf16 matmuls within 2e-2 tolerance"):
        const = ctx.enter_context(tc.tile_pool(name="const", bufs=1))
        wpool = ctx.enter_context(tc.tile_pool(name="wpool", bufs=1))
        big = ctx.enter_context(tc.tile_pool(name="big", bufs=1))
        pepool = ctx.enter_context(tc.tile_pool(name="pepool", bufs=5))
        avtpool = ctx.enter_context(tc.tile_pool(name="avtpool", bufs=2))
        npool = ctx.enter_context(tc.tile_pool(name="npool", bufs=2))
        ypool = ctx.enter_context(tc.tile_pool(name="ypool", bufs=2))
        # pair-end f32 snapshots of the av accumulators: frees the PSUM avA/avB
        # banks after ONE copy so the next pair's accumulation never stalls on
        # the (deferred) normalize chain.
        avsb = ctx.enter_context(tc.tile_pool(name="avsb", bufs=2))
        # PSUM budget (16KB/partition = 8 banks, exact fit):
        #   s    [128,2048] f32  4 banks  (scores: 2 kt x 2 heads per j)
        #   avA  [128, 512] f32  1 bank   (head A attention accumulator)
        #   avB  [128, 512] f32  1 bank
        #   y    [128, 512] f32  1 bank   (out-proj + phase1 rotation)
        #   misc [128, 512] f32  1 bank   (q-proj fillers, lrp, transposes)
        ps = ctx.enter_context(tc.tile_pool(name="ps", bufs=1, space="PSUM"))

        # ---- constants + weights: ONE dma_start per tensor, critical first --
        wk_sb = wpool.tile([128, ET, CK], BF16)
        nc.sync.dma_start(out=wk_sb[:], in_=wkd[:, :, :])

        # per-chunk tiles: whole-tile dependency tracking would otherwise gate
        # chunk-0 compute on chunk-3 DMA completion (and q-proj fillers on
        # in-flight score reads). xT is additionally split into et-halves so
        # the first kv matmuls start after half a chunk has landed.
        xh = {(sc, h): big.tile([128, 4, 512], BF16, tag=f"xT{sc}_{h}",
                                name=f"xT{sc}_{h}")
              for sc in range(SC) for h in range(2)}

        def xT_at(sc, et):
            return xh[(sc, et // 4)][:, et % 4, :]

        qT_ch = {sc: big.tile([128, QT, 512], BF16, tag=f"qT{sc}",
                              name=f"qT{sc}") for sc in range(SC)}
        kT_ch = {sc: big.tile([128, 512], BF16, tag=f"kT{sc}",
                              name=f"kT{sc}") for sc in range(SC)}
        vT_ch = {sc: big.tile([128, 512], BF16, tag=f"vT{sc}",
                              name=f"vT{sc}") for sc in range(SC)}
        # v natural + ones col: [kt-in-chunk, group, 65]
        vaug_ch = {sc: big.tile([128, KTC, 2, 65], BF16, tag=f"vaug{sc}",
                                name=f"vaug{sc}") for sc in range(SC)}

        # All input issues stay on the sync queue IN dependency order: a
        # second concurrent issue stream makes later transfers contend with
        # the critical x0 descriptors on the shared hardware DMA queues.
        # The first kv matmuls need exactly wk + x0a (+x0b), so those
        # transfer before wv/ident/wq.
        for h in range(2):
            nc.sync.dma_start(out=xh[(0, h)][:],
                              in_=xTd[:, 4 * h:4 * h + 4, 0:512])
        wv_sb = wpool.tile([128, ET, CK], BF16)
        nc.sync.dma_start(out=wv_sb[:], in_=wvd[:, :, :])
        ident = const.tile([128, 128], BF16)
        nc.sync.dma_start(out=ident[:], in_=identd[:, :])
        wq_sb = wpool.tile([128, ET, CQ], BF16)
        nc.sync.dma_start(out=wq_sb[:], in_=wqd[:, :, :])
        bq_sb = wpool.tile([128, QT], F32)
        nc.sync.dma_start(out=bq_sb[:], in_=bqd[:, :])
        ones_row = const.tile([1, 64], mybir.dt.float32r)
        nc.sync.dma_start(out=ones_row[:], in_=onesf[0:1, :])
        for sc in range(1, SC):
            for h in range(2):
                nc.sync.dma_start(
                    out=xh[(sc, h)][:],
                    in_=xTd[:, 4 * h:4 * h + 4, sc * 512:sc * 512 + 512])
        for sc in range(SC):
            nc.sync.dma_start(out=vaug_ch[sc][:, :, :, 64:65],
                              in_=onesd[:, 0:2 * KTC])
        wo_sb = wpool.tile([128, QT, E], BF16)
        nc.sync.dma_start(out=wo_sb[:], in_=wod[:, :, :])

        # ================= phase 1: K/V projections + transposes =================
        def kv_proj(sc):
            pk = ps.tile([128, 512], F32, tag="y", name=f"pk{sc}")
            for et in range(ET):
                nc.tensor.matmul(pk[:], wk_sb[:, et, :], xT_at(sc, et),
                                 start=(et == 0), stop=(et == ET - 1))
            nc.vector.tensor_copy(out=kT_ch[sc][:], in_=pk[:])
            pv = ps.tile([128, 512], F32, tag="misc", name=f"pv{sc}")
            for et in range(ET):
                nc.tensor.matmul(pv[:], wv_sb[:, et, :], xT_at(sc, et),
                                 start=(et == 0), stop=(et == ET - 1))
            nc.vector.tensor_copy(out=vT_ch[sc][:], in_=pv[:])
            for j in range(KTC):
                ptr = ps.tile([128, 128], BF16, tag="y", name=f"ptr{sc}_{j}")
                nc.tensor.transpose(ptr[:], vT_ch[sc][:, j * 128:(j + 1) * 128],
                                    ident[:])
                # both groups' 64 dims in one strided copy
                nc.vector.tensor_copy(out=vaug_ch[sc][:, j, :, 0:64],
                                      in_=ptr[:, :])



        # ---- helpers issued inline or as fillers ----
        def qproj(sc, t):
            """One qT tile: 8-matmul accumulation + bias, issued atomically
            (tag-rotation safety: nothing else may allocate this tag between
            a tile's first write and its last read)."""
            pq = ps.tile([128, 512], F32, tag="misc", name=f"pq{sc}_{t}")
            for et in range(ET):
                nc.tensor.matmul(pq[:], wq_sb[:, et, t * 128:(t + 1) * 128],
                                 xT_at(sc, et),
                                 start=(et == 0), stop=(et == ET - 1))
            nc.scalar.add(out=qT_ch[sc][:, t, :], in_=pq[:],
                          add=bq_sb[:, t:t + 1])

        def outproj_et(qc, et, avT_t):
            lo = qc * 512
            # alternate PSUM banks so consecutive out-proj tiles never
            # serialize on a single bank's WAR against the ysb copy
            yp = ps.tile([128, 512], F32, tag=("y" if et % 2 == 0 else "misc"),
                         name=f"yp{qc}_{et}")
            for t in range(QT):
                nc.tensor.matmul(yp[:], wo_sb[:, t, et * 128:(et + 1) * 128],
                                 avT_t[:, t, :], start=(t == 0), stop=(t == QT - 1))
            ysb = ypool.tile([128, 512], BF16, tag="ysb", name=f"ysb{qc}_{et}")
            # alternate the PSUM->SBUF copy between ACT and DVE to balance
            if et % 2 == 0:
                nc.scalar.copy(out=ysb[:], in_=yp[:])
            else:
                nc.vector.tensor_copy(out=ysb[:], in_=yp[:])
            nc.gpsimd.dma_start(out=yT[et * 128:(et + 1) * 128, lo:lo + 512],
                                in_=ysb[:])

        # Phase 1: interleave chunk-0 q-projection (needs only x0) between the
        # kv projections so the PE keeps working while x1-x3 DMAs land.
        # kv(2)/kv(3) are NOT issued here: pair 0 only needs chunk 0-1 keys
        # for its first 8 kt, so they run as its first fillers, inside the
        # exp shadow where the PE would otherwise idle.
        kv_proj(0)
        qproj(0, 0)
        qproj(0, 1)
        kv_proj(1)
        qproj(0, 2)
        qproj(0, 3)

        # ================= phase 2: attention, software-pipelined =================
        def normalize(dens, sbs, avT_t, p, qc, last=False):
            """avT = av[0:64] * recip(av[64]) from the pair-end av snapshots.
            The denominator row (already copied to partition 0 at pair end) is
            replicated into a [64,512] PSUM span via a K=1 f32r matmul, DVE
            computes the reciprocal, and the final multiply runs on the idle
            GPSIMD engine so the DVE burst at pair start stays short (a long
            DVE queue here delays Schraudolph exps and stalls the AV stream).
            The very last normalize gates the final out-projection, so its
            multiply runs on the faster DVE instead (GPSIMD is ~2x slower;
            a GPSIMD partition_broadcast variant that removed the lrp matmul
            entirely measured 600us+ — the broadcast is several us in-flow
            and serializes the avT chain)."""
            for g, lrptag in ((0, "y"), (1, "misc")):
                lrp = ps.tile([128, 512], F32, tag=lrptag, name=f"lrp{qc}_{p}_{g}")
                nc.tensor.matmul(lrp[0:64, :], ones_row[:], dens[g][:],
                                 start=True, stop=True)
                rinv = npool.tile([64, 512], F32, tag="rinv",
                                  name=f"rinv{qc}_{p}_{g}")
                nc.vector.reciprocal_approx_fast(out=rinv[:], in_=lrp[0:64, :])
                eng = nc.vector if last else nc.gpsimd
                eng.tensor_tensor(
                    out=avT_t[g * 64:g * 64 + 64, p, :], in0=sbs[g][0:64, :],
                    in1=rinv[:], op=MUL)

        avT_tiles = {}
        pending_norm = deque()  # deferred normalizes, popped after next S/exp
        for qc in range(SC):
            fillers = deque()
            if qc == 0:
                # chunk 2/3 K/V projections: popped at kt=5 and kt=7 of
                # pair 0, comfortably before their keys are needed (kt 8/12)
                fillers.append(lambda: kv_proj(2))
                fillers.append(lambda: kv_proj(3))
            if qc + 1 < SC:
                for t in range(QT):
                    fillers.append(lambda t=t, s=qc + 1: qproj(s, t))
            if qc - 1 >= 0:
                prev_avT = avT_tiles[qc - 1]
                for et in range(ET):
                    fillers.append(lambda et=et, a=prev_avT, s=qc - 1:
                                   outproj_et(s, et, a))

            avT_t = avtpool.tile([128, QT, 512], BF16, tag="avT", name=f"avT{qc}")
            avT_tiles[qc] = avT_t
            for p in range(QT):
                # scores/exp run two kt ahead of AV so exp latency never
                # stalls the PE; pair (p-1)'s normalize is spliced in after
                # this pair's first exp so its PE/DVE ops hide under the exp
                # stream instead of stalling the pair boundary.
                avpA = avpB = None
                peA_t, peB_t = {}, {}

                def av_step(pkt):
                    c, j = pkt // KTC, pkt % KTC
                    pp = peA_t.pop(pkt)
                    peB_t.pop(pkt)
                    nc.tensor.matmul(
                        avpA[0:65, :], vaug_ch[c][:, j, 0, :], pp[:, 0:512],
                        start=(pkt == 0), stop=(pkt == KT - 1))
                    nc.tensor.matmul(
                        avpB[0:65, :], vaug_ch[c][:, j, 1, :], pp[:, 512:1024],
                        start=(pkt == 0), stop=(pkt == KT - 1))

                for kt in range(KT):
                    c, j = kt // KTC, kt % KTC
                    sT = ps.tile([128, 1024], F32, tag="s", bufs=2,
                                 name=f"s{qc}_{p}_{kt}")
                    nc.tensor.matmul(
                        sT[:, 0:512],
                        kT_ch[c][0:64, j * 128:(j + 1) * 128],
                        qT_ch[qc][0:64, p, :], start=True, stop=True)
                    nc.tensor.matmul(
                        sT[:, 512:1024],
                        kT_ch[c][64:128, j * 128:(j + 1) * 128],
                        qT_ch[qc][64:128, p, :], start=True, stop=True)
                    # one whole-tile exp per kt, alternating engines: a single
                    # [128,1024] instruction amortizes the ~230-cycle fixed
                    # cost that two half-tile instructions pay twice, and the
                    # per-engine rate drops to ~550-600ns/kt so neither exp
                    # stream is ever co-critical with the PE again.
                    pe_t = pepool.tile([128, 1024], BF16, tag="pe",
                                       name=f"pe{qc}_{p}_{kt}")
                    if kt % 2 == 1:
                        nc.vector.tensor_scalar(
                            out=pe_t.bitcast(mybir.dt.int16), in0=sT[:],
                            scalar1=SCHR_A, scalar2=SCHR_B, op0=MUL, op1=ADD)
                    else:
                        nc.scalar.activation(pe_t[:], sT[:], EXP,
                                             scale=float(SCALE))
                    peA_t[kt] = pe_t
                    peB_t[kt] = pe_t
                    if kt == 1:
                        while pending_norm:
                            pending_norm.popleft()()
                        # allocate accumulators after the deferred normalize of
                        # the previous pair has issued its reads (bufs=1 slots)
                        avpA = ps.tile([128, 512], F32, tag="avA",
                                       name=f"avpA{qc}_{p}")
                        avpB = ps.tile([128, 512], F32, tag="avB",
                                       name=f"avpB{qc}_{p}")
                    # batch-2: two kts of scores, then a 4-MM AV block. The PE
                    # array pays ~100ns whenever a matmul follows one with a
                    # different tiling config (row-tiled scores vs full-array
                    # AV), so grouping halves the transition count.
                    if kt >= 3 and kt % 2 == 1:
                        av_step(kt - 3)
                        av_step(kt - 2)
                        # pop fillers right after an AV block: the filler MMs
                        # are full-array like AV, so the transition is free.
                        if kt >= 5 and fillers:
                            fillers.popleft()()
                av_step(KT - 2)
                av_step(KT - 1)
                # snapshot the accumulators to SBUF: the avA/avB banks are
                # freed after these single reads, so the next pair's av_step(0)
                # never waits on the normalize chain. The den row is staged to
                # partition 0 here too, so the deferred lrp matmul is
                # immediately ready when popped.
                sbs, dens = [], []
                for g, avp in ((0, avpA), (1, avpB)):
                    sb = avsb.tile([65, 512], F32, tag=f"sb{g}",
                                   name=f"sb{qc}_{p}_{g}")
                    # PSUM->SBUF snapshot on ACT: keeps the DVE queue short so
                    # the next pair's Schraudolph exps start immediately, and
                    # the avp bank is freed by this single read (a den copy
                    # from PSUM on DVE would hold the WAR hostage to the DVE
                    # exp queue depth - measured +5us).
                    nc.scalar.copy(out=sb[:], in_=avp[0:65, :])
                    den = npool.tile([1, 512], mybir.dt.float32r, tag=f"den{g}",
                                     name=f"den{qc}_{p}_{g}")
                    nc.vector.tensor_copy(out=den[:], in_=sb[64:65, :])
                    sbs.append(sb)
                    dens.append(den)
                pending_norm.append(
                    lambda d=dens, s=sbs, t=avT_t, p=p, q=qc, **kw:
                    normalize(d, s, t, p, q, **kw))
            while fillers:
                fillers.popleft()()
        while pending_norm:
            pending_norm.popleft()(last=True)
        # out-projection for the last chunk
        for et in range(ET):
            outproj_et(SC - 1, et, avT_tiles[SC - 1])
    nc.compile()
    return nc


def _shard_inputs(x, Wq, bq, Wk, bk, Wv, bv, Wo, bo):
    """Build the 8 per-core input maps, host-prepacked to SBUF layouts."""
    x = np.asarray(x, dtype=np.float32)

    def to_sb(a, np_dtype):
        # [E_rows, F] -> [128, E_rows//128, F] SBUF layout
        r, f = a.shape
        return np.ascontiguousarray(
            a.reshape(r // 128, 128, f).transpose(1, 0, 2)).astype(np_dtype)

    in_maps = []
    for c in range(8):
        b, H = c // 2, c % 2
        heads = [8 * H + t for t in range(4)] + [8 * H + t + 4 for t in range(4)]
        # qT tile t holds (local head t -> partitions 0-63, local head t+4 -> 64-127)
        order = []
        for t in range(4):
            order.extend(range(heads[t] * 64, heads[t] * 64 + 64))
            order.extend(range(heads[t + 4] * 64, heads[t + 4] * 64 + 64))
        order = np.asarray(order)
        wq_p = to_sb(np.asarray(Wq, np.float32)[:, order], NPBF16)
        bq_p = np.ascontiguousarray(
            np.asarray(bq, np.float32)[order].reshape(4, 128).T)
        wo_p = to_sb(np.asarray(Wo, np.float32)[order, :], NPBF16)
        wk_s = to_sb(np.asarray(Wk, np.float32)[:, H * 128:(H + 1) * 128], NPBF16)
        wv_s = to_sb(np.asarray(Wv, np.float32)[:, H * 128:(H + 1) * 128], NPBF16)
        xT_b = to_sb(x[b].T, NPBF16)
        in_maps.append({
            "xTd": xT_b, "wqd": wq_p, "wkd": wk_s, "wvd": wv_s, "wod": wo_p,
            "bqd": bq_p,
            "identd": np.eye(128, dtype=NPBF16),
            "onesd": np.ones((128, 64), dtype=NPBF16),
            "onesf": np.ones((1, 64), dtype=np.float32),
        })
    return in_maps


def kernel(x, Wq, bq, Wk, bk, Wv, bv, Wo, bo, _trace=False):
    if "nc" not in _NC_CACHE:
        _NC_CACHE["nc"] = build_nc()
    nc = _NC_CACHE["nc"]
    in_maps = _shard_inputs(x, Wq, bq, Wk, bk, Wv, bv, Wo, bo)
    res = run_bass_kernel_spmd(nc, in_maps, list(range(8)), trace=_trace)
    # bk is a per-query constant shift inside softmax -> cancels exactly.
    # bv contributes bv_exp @ Wo to every output row (softmax weights sum to
    # 1), where bv_exp[e] = bv[g*64 + d] for e = g*256 + r*64 + d.
    bo = np.asarray(bo, dtype=np.float32)
    bv = np.asarray(bv, dtype=np.float32)
    Wo_f = np.asarray(Wo, dtype=np.float32)
    g_idx = np.arange(E) // 256
    d_idx = np.arange(E) % 64
    bv_exp = bv[g_idx * 64 + d_idx]
    bo_eff = bo + bv_exp @ Wo_f
    out = np.empty((B, S, E), dtype=np.float32)
    for b in range(B):
        yTb = (res.results[2 * b]["yT"].astype(np.float32)
               + res.results[2 * b + 1]["yT"].astype(np.float32))
        out[b] = yTb.T + bo_eff
    if _trace:
        return out, res
    return out


# revision 54
# speedup vs baseline: 1.1887x; 1.1887x over previous
"""GQA self-attention kernel for Trainium2, sharded over 8 NeuronCores.

Problem: x[4, 2048, 1024], 16 heads / 4 KV groups / head_dim 64.
Sharding: batch (4) x head-half (2 KV groups each) = 8 cores.

Final dataflow (all-bf16 PE path, transposed world: features on partitions):
  xT[1024,2048]b16 -> qT[512,2048], kT[128,2048], vT[128,2048]  (PE, bf16)
  vT --PE transpose--> vaug[seq,65] tiles (ones col appended -> softmax sums)
  scores s[k,q] = kT_g^T(d,kpos) . qT_h(d,q): K=64 contraction, the two KV
    groups run CONCURRENTLY in the upper/lower 64-row halves of the PE array
    (row tiling via base_partition-derived tile_position)
  exp: ONE whole [128,1024] instruction per kt tile, alternating engines —
    even kts on ACT (exact Exp), odd kts on DVE (Schraudolph fast-exp
    emitting bf16 bits). The single wide instruction amortizes each
    engine's ~230-cycle fixed cost, and alternation keeps both exp streams
    at ~550-600ns/kt, safely under the PE's ~870ns/kt.
  av[65,q] += vaug^T p (row 64 = softmax denominator); AV lags the score/exp
    stream by 2-3 kt tiles (two kts of scores, then a 4-matmul AV block —
    grouping same-tiling-config matmuls halves the ~100ns weight-switch
    transition cost) so exp latency is fully hidden.
  pair end: av snapshot PSUM->SBUF on ACT (frees the PSUM accumulator banks
    immediately); normalize avT = av[0:64] * recip(av[64]) is deferred into
    the next pair's exp shadow (denom row replicated via K=1 f32r MM, DVE
    reciprocal, multiply on the otherwise-idle GPSIMD engine).
  yT[e,q] = Wo_p^T . avT_norm -> DRAM (bf16; host upcasts + sums partials)
The attention loop is software-pipelined; Q-projection (next chunk) and
out-projection (previous chunk) matmuls are spliced into the PE queue as
fillers so the PE stays dense (HAM stays warm).

Bias handling (exact): bk cancels in softmax (per-query constant shift);
bv is folded into the host-side output bias (bo_eff = bo + bv_exp @ Wo);
bq is applied on-device as before.

Startup: all inputs are host-prepacked into their SBUF layouts so each
tensor loads with ONE dma_start (the sync-queue DIRECT2D issue path costs
~650ns per dma_start; the v2 kernel's 97 input issues serialized ~60us).
All chunk-striped SBUF tensors (xT, kT, vT, vaug) are per-chunk tiles so
whole-tile dependency tracking never gates chunk-0 compute on chunk-3 DMAs.
Host: y[b] = (yT[2b] + yT[2b+1]).T + bo_eff
"""

import os
import sys
import numpy as np

# Coarse whole-tile dependency tracking: subtile AP-overlap analysis has
# proven racy for this kernel's 3D-sliced persistent tiles (intermittent
# first-run corruption); whole-tile deps are conservative and safe.
os.environ["BY_DEFAULT_DISABLE_SUBTILE_DEPS"] = "1"

sys.path.insert(0, "/opt/trn_rl_repo")

from collections import deque
from contextlib import ExitStack

import ml_dtypes

import concourse.bass as bass
import concourse.bacc as bacc
import concourse.mybir as mybir
from concourse import tile
from concourse.bass_utils import run_bass_kernel_spmd

F32 = mybir.dt.float32
BF16 = mybir.dt.bfloat16
NPBF16 = ml_dtypes.bfloat16

B, S, E = 4, 2048, 1024
NUM_HEADS, NUM_GROUPS, D = 16, 4, 64
CQ = 512          # q cols per core (8 heads)
CK = 128          # kv cols per core (2 groups)
ET = E // 128     # 8 embed K-tiles
SC = S // 512     # 4 seq chunks of 512
KT = S // 128     # 16 key tiles of 128
KTC = KT // SC    # 4 key tiles per chunk
QT = CQ // 128    # 4 qT partition tiles (pair p: head p @0-63, head p+4 @64-127)
SCALE = 1.0 / np.sqrt(np.float32(D))
# Schraudolph fast-exp constants emitting bf16 bits as int16:
#   bf16_bits(e^(x*SCALE)) ~= int16((2^23/ln2)*SCALE/2^16 * x + (127*2^23-c)/2^16)
# c = 486411 minimizes rms relative error (~1.8%); odd kt tiles use it on
# DVE while even kts use exact Exp on ACT — 50% of p-elements through the
# approximation gives sim/HW rel err 9.7e-3 (tolerance 2e-2).
SCHR_A = float((2.0 ** 23 / np.log(2.0)) * SCALE / 65536.0)
SCHR_B = float((127 * 2 ** 23 - 486411) / 65536.0)

_NC_CACHE = {}


def build_nc():
    nc = bacc.Bacc(None, target_bir_lowering=False)

    # All tensors host-prepacked to SBUF layout: one dma_start each.
    xTd = nc.dram_tensor("xTd", [128, ET, S], BF16, kind="ExternalInput")
    wqd = nc.dram_tensor("wqd", [128, ET, CQ], BF16, kind="ExternalInput")
    wkd = nc.dram_tensor("wkd", [128, ET, CK], BF16, kind="ExternalInput")
    wvd = nc.dram_tensor("wvd", [128, ET, CK], BF16, kind="ExternalInput")
    wod = nc.dram_tensor("wod", [128, QT, E], BF16, kind="ExternalInput")
    bqd = nc.dram_tensor("bqd", [128, QT], F32, kind="ExternalInput")
    identd = nc.dram_tensor("identd", [128, 128], BF16, kind="ExternalInput")
    onesd = nc.dram_tensor("onesd", [128, 64], BF16, kind="ExternalInput")
    onesf = nc.dram_tensor("onesf", [1, 64], mybir.dt.float32r, kind="ExternalInput")
    # bf16 output halves the 8MB/core store traffic; the host upcasts and
    # sums the two head-half partials in f32 (adds ~0.1% quantization noise,
    # well within the 2e-2 budget).
    yT = nc.dram_tensor("yT", [E, S], BF16, kind="ExternalOutput")

    ADD = mybir.AluOpType.add
    MUL = mybir.AluOpType.mult
    EXP = mybir.ActivationFunctionType.Exp

    with tile.TileContext(nc) as tc, ExitStack() as ctx, \
            nc.allow_low_precision(reason="bf16 matmuls within 2e-2 tolerance"):
        const = ctx.enter_context(tc.tile_pool(name="const", bufs=1))
        wpool = ctx.enter_context(tc.tile_pool(name="wpool", bufs=1))
        big = ctx.enter_context(tc.tile_pool(name="big", bufs=1))
        pepool = ctx.enter_context(tc.tile_pool(name="pepool", bufs=5))
        avtpool = ctx.enter_context(tc.tile_pool(name="avtpool", bufs=2))
        npool = ctx.enter_context(tc.tile_pool(name="npool", bufs=2))
        ypool = ctx.enter_context(tc.tile_pool(name="ypool", bufs=2))
        # pair-end f32 snapshots of the av accumulators: frees the PSUM avA/avB
        # banks after ONE copy so the next pair's accumulation never stalls on
        # the (deferred) normalize chain.
        avsb = ctx.enter_context(tc.tile_pool(name="avsb", bufs=2))
        # PSUM budget (16KB/partition = 8 banks, exact fit):
        #   s    [128,2048] f32  4 banks  (scores: 2 kt x 2 heads per j)
        #   avA  [128, 512] f32  1 bank   (head A attention accumulator)
        #   avB  [128, 512] f32  1 bank
        #   y    [128, 512] f32  1 bank   (out-proj + phase1 rotation)
        #   misc [128, 512] f32  1 bank   (q-proj fillers, lrp, transposes)
        ps = ctx.enter_context(tc.tile_pool(name="ps", bufs=1, space="PSUM"))

        # ---- constants + weights: ONE dma_start per tensor, critical first --
        wk_sb = wpool.tile([128, ET, CK], BF16)
        nc.sync.dma_start(out=wk_sb[:], in_=wkd[:, :, :])
        wv_sb = wpool.tile([128, ET, CK], BF16)
        nc.sync.dma_start(out=wv_sb[:], in_=wvd[:, :, :])
        ident = const.tile([128, 128], BF16)
        nc.sync.dma_start(out=ident[:], in_=identd[:, :])
        ones_row = const.tile([1, 64], mybir.dt.float32r)
        nc.sync.dma_start(out=ones_row[:], in_=onesf[0:1, :])

        # per-chunk tiles: whole-tile dependency tracking would otherwise gate
        # chunk-0 compute on chunk-3 DMA completion (and q-proj fillers on
        # in-flight score reads). xT is additionally split into et-halves so
        # the first kv matmuls start after half a chunk has landed.
        xh = {(sc, h): big.tile([128, 4, 512], BF16, tag=f"xT{sc}_{h}",
                                name=f"xT{sc}_{h}")
              for sc in range(SC) for h in range(2)}

        def xT_at(sc, et):
            return xh[(sc, et // 4)][:, et % 4, :]

        qT_ch = {sc: big.tile([128, QT, 512], BF16, tag=f"qT{sc}",
                              name=f"qT{sc}") for sc in range(SC)}
        kT_ch = {sc: big.tile([128, 512], BF16, tag=f"kT{sc}",
                              name=f"kT{sc}") for sc in range(SC)}
        vT_ch = {sc: big.tile([128, 512], BF16, tag=f"vT{sc}",
                              name=f"vT{sc}") for sc in range(SC)}
        # v natural + ones col: [kt-in-chunk, group, 65]
        vaug_ch = {sc: big.tile([128, KTC, 2, 65], BF16, tag=f"vaug{sc}",
                                name=f"vaug{sc}") for sc in range(SC)}

        # All input issues stay on the sync queue IN dependency order: a
        # second concurrent issue stream makes later transfers contend with
        # the critical x0 descriptors on the shared hardware DMA queues.
        # (Reordering wv/ident behind x0 measured +70us — the downstream
        # schedule is extremely sensitive to arrival order; keep as-is.)
        for h in range(2):
            nc.sync.dma_start(out=xh[(0, h)][:],
                              in_=xTd[:, 4 * h:4 * h + 4, 0:512])
        wq_sb = wpool.tile([128, ET, CQ], BF16)
        nc.sync.dma_start(out=wq_sb[:], in_=wqd[:, :, :])
        bq_sb = wpool.tile([128, QT], F32)
        nc.sync.dma_start(out=bq_sb[:], in_=bqd[:, :])
        for sc in range(1, SC):
            for h in range(2):
                nc.sync.dma_start(
                    out=xh[(sc, h)][:],
                    in_=xTd[:, 4 * h:4 * h + 4, sc * 512:sc * 512 + 512])
        for sc in range(SC):
            nc.sync.dma_start(out=vaug_ch[sc][:, :, :, 64:65],
                              in_=onesd[:, 0:2 * KTC])
        wo_sb = wpool.tile([128, QT, E], BF16)
        nc.sync.dma_start(out=wo_sb[:], in_=wod[:, :, :])

        # ================= phase 1: K/V projections + transposes =================
        def kv_proj(sc):
            pk = ps.tile([128, 512], F32, tag="y", name=f"pk{sc}")
            for et in range(ET):
                nc.tensor.matmul(pk[:], wk_sb[:, et, :], xT_at(sc, et),
                                 start=(et == 0), stop=(et == ET - 1))
            nc.vector.tensor_copy(out=kT_ch[sc][:], in_=pk[:])
            pv = ps.tile([128, 512], F32, tag="misc", name=f"pv{sc}")
            for et in range(ET):
                nc.tensor.matmul(pv[:], wv_sb[:, et, :], xT_at(sc, et),
                                 start=(et == 0), stop=(et == ET - 1))
            nc.vector.tensor_copy(out=vT_ch[sc][:], in_=pv[:])
            for j in range(KTC):
                ptr = ps.tile([128, 128], BF16, tag="y", name=f"ptr{sc}_{j}")
                nc.tensor.transpose(ptr[:], vT_ch[sc][:, j * 128:(j + 1) * 128],
                                    ident[:])
                # both groups' 64 dims in one strided copy
                nc.vector.tensor_copy(out=vaug_ch[sc][:, j, :, 0:64],
                                      in_=ptr[:, :])



        # ---- helpers issued inline or as fillers ----
        def qproj(sc, t):
            """One qT tile: 8-matmul accumulation + bias, issued atomically
            (tag-rotation safety: nothing else may allocate this tag between
            a tile's first write and its last read)."""
            pq = ps.tile([128, 512], F32, tag="misc", name=f"pq{sc}_{t}")
            for et in range(ET):
                nc.tensor.matmul(pq[:], wq_sb[:, et, t * 128:(t + 1) * 128],
                                 xT_at(sc, et),
                                 start=(et == 0), stop=(et == ET - 1))
            nc.scalar.add(out=qT_ch[sc][:, t, :], in_=pq[:],
                          add=bq_sb[:, t:t + 1])

        def outproj_et(qc, et, avT_t):
            lo = qc * 512
            # alternate PSUM banks so consecutive out-proj tiles never
            # serialize on a single bank's WAR against the ysb copy
            yp = ps.tile([128, 512], F32, tag=("y" if et % 2 == 0 else "misc"),
                         name=f"yp{qc}_{et}")
            for t in range(QT):
                nc.tensor.matmul(yp[:], wo_sb[:, t, et * 128:(et + 1) * 128],
                                 avT_t[:, t, :], start=(t == 0), stop=(t == QT - 1))
            ysb = ypool.tile([128, 512], BF16, tag="ysb", name=f"ysb{qc}_{et}")
            # alternate the PSUM->SBUF copy between ACT and DVE to balance
            if et % 2 == 0:
                nc.scalar.copy(out=ysb[:], in_=yp[:])
            else:
                nc.vector.tensor_copy(out=ysb[:], in_=yp[:])
            nc.gpsimd.dma_start(out=yT[et * 128:(et + 1) * 128, lo:lo + 512],
                                in_=ysb[:])

        # Phase 1: interleave chunk-0 q-projection (needs only x0) between the
        # kv projections so the PE keeps working while x1-x3 DMAs land.
        # kv(2)/kv(3) are NOT issued here: pair 0 only needs chunk 0-1 keys
        # for its first 8 kt, so they run as its first fillers, inside the
        # exp shadow where the PE would otherwise idle.
        kv_proj(0)
        qproj(0, 0)
        qproj(0, 1)
        kv_proj(1)
        qproj(0, 2)
        qproj(0, 3)

        # ================= phase 2: attention, software-pipelined =================
        def normalize(dens, sbs, avT_t, p, qc, last=False):
            """avT = av[0:64] * recip(av[64]) from the pair-end av snapshots.
            The denominator row (already copied to partition 0 at pair end) is
            replicated into a [64,512] PSUM span via a K=1 f32r matmul, DVE
            computes the reciprocal, and the final multiply runs on the idle
            GPSIMD engine so the DVE burst at pair start stays short (a long
            DVE queue here delays Schraudolph exps and stalls the AV stream).
            The very last normalize gates the final out-projection, so its
            multiply runs on the faster DVE instead (GPSIMD is ~2x slower;
            a GPSIMD partition_broadcast variant that removed the lrp matmul
            entirely measured 600us+ — the broadcast is several us in-flow
            and serializes the avT chain)."""
            for g, lrptag in ((0, "y"), (1, "misc")):
                lrp = ps.tile([128, 512], F32, tag=lrptag, name=f"lrp{qc}_{p}_{g}")
                nc.tensor.matmul(lrp[0:64, :], ones_row[:], dens[g][:],
                                 start=True, stop=True)
                rinv = npool.tile([64, 512], F32, tag="rinv",
                                  name=f"rinv{qc}_{p}_{g}")
                nc.vector.reciprocal_approx_fast(out=rinv[:], in_=lrp[0:64, :])
                eng = nc.vector if last else nc.gpsimd
                eng.tensor_tensor(
                    out=avT_t[g * 64:g * 64 + 64, p, :], in0=sbs[g][0:64, :],
                    in1=rinv[:], op=MUL)

        avT_tiles = {}
        pending_norm = deque()  # deferred normalizes, popped after next S/exp
        for qc in range(SC):
            fillers = deque()
            if qc == 0:
                # chunk 2/3 K/V projections: popped at kt=5 and kt=7 of
                # pair 0, comfortably before their keys are needed (kt 8/12)
                fillers.append(lambda: kv_proj(2))
                fillers.append(lambda: kv_proj(3))
            if qc + 1 < SC:
                for t in range(QT):
                    fillers.append(lambda t=t, s=qc + 1: qproj(s, t))
            if qc - 1 >= 0:
                prev_avT = avT_tiles[qc - 1]
                for et in range(ET):
                    fillers.append(lambda et=et, a=prev_avT, s=qc - 1:
                                   outproj_et(s, et, a))

            avT_t = avtpool.tile([128, QT, 512], BF16, tag="avT", name=f"avT{qc}")
            avT_tiles[qc] = avT_t
            for p in range(QT):
                # scores/exp run two kt ahead of AV so exp latency never
                # stalls the PE; pair (p-1)'s normalize is spliced in after
                # this pair's first exp so its PE/DVE ops hide under the exp
                # stream instead of stalling the pair boundary.
                avpA = avpB = None
                peA_t, peB_t = {}, {}

                def av_step(pkt):
                    c, j = pkt // KTC, pkt % KTC
                    pp = peA_t.pop(pkt)
                    peB_t.pop(pkt)
                    nc.tensor.matmul(
                        avpA[0:65, :], vaug_ch[c][:, j, 0, :], pp[:, 0:512],
                        start=(pkt == 0), stop=(pkt == KT - 1))
                    nc.tensor.matmul(
                        avpB[0:65, :], vaug_ch[c][:, j, 1, :], pp[:, 512:1024],
                        start=(pkt == 0), stop=(pkt == KT - 1))

                for kt in range(KT):
                    c, j = kt // KTC, kt % KTC
                    sT = ps.tile([128, 1024], F32, tag="s", bufs=2,
                                 name=f"s{qc}_{p}_{kt}")
                    nc.tensor.matmul(
                        sT[:, 0:512],
                        kT_ch[c][0:64, j * 128:(j + 1) * 128],
                        qT_ch[qc][0:64, p, :], start=True, stop=True)
                    nc.tensor.matmul(
                        sT[:, 512:1024],
                        kT_ch[c][64:128, j * 128:(j + 1) * 128],
                        qT_ch[qc][64:128, p, :], start=True, stop=True)
                    # one whole-tile exp per kt, alternating engines: a single
                    # [128,1024] instruction amortizes the ~230-cycle fixed
                    # cost that two half-tile instructions pay twice, and the
                    # per-engine rate drops to ~550-600ns/kt so neither exp
                    # stream is ever co-critical with the PE again.
                    pe_t = pepool.tile([128, 1024], BF16, tag="pe",
                                       name=f"pe{qc}_{p}_{kt}")
                    if kt % 2 == 1:
                        nc.vector.tensor_scalar(
                            out=pe_t.bitcast(mybir.dt.int16), in0=sT[:],
                            scalar1=SCHR_A, scalar2=SCHR_B, op0=MUL, op1=ADD)
                    else:
                        nc.scalar.activation(pe_t[:], sT[:], EXP,
                                             scale=float(SCALE))
                    peA_t[kt] = pe_t
                    peB_t[kt] = pe_t
                    if kt == 1:
                        while pending_norm:
                            pending_norm.popleft()()
                        # allocate accumulators after the deferred normalize of
                        # the previous pair has issued its reads (bufs=1 slots)
                        avpA = ps.tile([128, 512], F32, tag="avA",
                                       name=f"avpA{qc}_{p}")
                        avpB = ps.tile([128, 512], F32, tag="avB",
                                       name=f"avpB{qc}_{p}")
                    # batch-2: two kts of scores, then a 4-MM AV block. The PE
                    # array pays ~100ns whenever a matmul follows one with a
                    # different tiling config (row-tiled scores vs full-array
                    # AV), so grouping halves the transition count.
                    if kt >= 3 and kt % 2 == 1:
                        av_step(kt - 3)
                        av_step(kt - 2)
                        # pop fillers right after an AV block: the filler MMs
                        # are full-array like AV, so the transition is free.
                        if kt >= 5 and fillers:
                            fillers.popleft()()
                av_step(KT - 2)
                av_step(KT - 1)
                # snapshot the accumulators to SBUF: the avA/avB banks are
                # freed after these single reads, so the next pair's av_step(0)
                # never waits on the normalize chain. The den row is staged to
                # partition 0 here too, so the deferred lrp matmul is
                # immediately ready when popped.
                sbs, dens = [], []
                for g, avp in ((0, avpA), (1, avpB)):
                    sb = avsb.tile([65, 512], F32, tag=f"sb{g}",
                                   name=f"sb{qc}_{p}_{g}")
                    # PSUM->SBUF snapshot on ACT: keeps the DVE queue short so
                    # the next pair's Schraudolph exps start immediately, and
                    # the avp bank is freed by this single read (a den copy
                    # from PSUM on DVE would hold the WAR hostage to the DVE
                    # exp queue depth - measured +5us).
                    nc.scalar.copy(out=sb[:], in_=avp[0:65, :])
                    den = npool.tile([1, 512], mybir.dt.float32r, tag=f"den{g}",
                                     name=f"den{qc}_{p}_{g}")
                    nc.vector.tensor_copy(out=den[:], in_=sb[64:65, :])
                    sbs.append(sb)
                    dens.append(den)
                pending_norm.append(
                    lambda d=dens, s=sbs, t=avT_t, p=p, q=qc, **kw:
                    normalize(d, s, t, p, q, **kw))
            while fillers:
                fillers.popleft()()
        while pending_norm:
            pending_norm.popleft()(last=True)
        # out-projection for the last chunk
        for et in range(ET):
            outproj_et(SC - 1, et, avT_tiles[SC - 1])
    nc.compile()
    return nc


def _shard_inputs(x, Wq, bq, Wk, bk, Wv, bv, Wo, bo):
    """Build the 8 per-core input maps, host-prepacked to SBUF layouts."""
    x = np.asarray(x, dtype=np.float32)

    def to_sb(a, np_dtype):
        # [E_rows, F] -> [128, E_rows//128, F] SBUF layout
        r, f = a.shape
        return np.ascontiguousarray(
            a.reshape(r // 128, 128, f).transpose(1, 0, 2)).astype(np_dtype)

    in_maps = []
    for c in range(8):
        b, H = c // 2, c % 2
        heads = [8 * H + t for t in range(4)] + [8 * H + t + 4 for t in range(4)]
        # qT tile t holds (local head t -> partitions 0-63, local head t+4 -> 64-127)
        order = []
        for t in range(4):
            order.extend(range(heads[t] * 64, heads[t] * 64 + 64))
            order.extend(range(heads[t + 4] * 64, heads[t + 4] * 64 + 64))
        order = np.asarray(order)
        wq_p = to_sb(np.asarray(Wq, np.float32)[:, order], NPBF16)
        bq_p = np.ascontiguousarray(
            np.asarray(bq, np.float32)[order].reshape(4, 128).T)
        wo_p = to_sb(np.asarray(Wo, np.float32)[order, :], NPBF16)
        wk_s = to_sb(np.asarray(Wk, np.float32)[:, H * 128:(H + 1) * 128], NPBF16)
        wv_s = to_sb(np.asarray(Wv, np.float32)[:, H * 128:(H + 1) * 128], NPBF16)
        xT_b = to_sb(x[b].T, NPBF16)
        in_maps.append({
            "xTd": xT_b, "wqd": wq_p, "wkd": wk_s, "wvd": wv_s, "wod": wo_p,
            "bqd": bq_p,
            "identd": np.eye(128, dtype=NPBF16),
            "onesd": np.ones((128, 64), dtype=NPBF16),
            "onesf": np.ones((1, 64), dtype=np.float32),
        })
    return in_maps


def kernel(x, Wq, bq, Wk, bk, Wv, bv, Wo, bo, _trace=False):
    if "nc" not in _NC_CACHE:
        _NC_CACHE["nc"] = build_nc()
    nc = _NC_CACHE["nc"]
    in_maps = _shard_inputs(x, Wq, bq, Wk, bk, Wv, bv, Wo, bo)
    res = run_bass_kernel_spmd(nc, in_maps, list(range(8)), trace=_trace)
    # bk is a per-query constant shift inside softmax -> cancels exactly.
    # bv contributes bv_exp @ Wo to every output row (softmax weights sum to
    # 1), where bv_exp[e] = bv[g*64 + d] for e = g*256 + r*64 + d.
    bo = np.asarray(bo, dtype=np.float32)
    bv = np.asarray(bv, dtype=np.float32)
    Wo_f = np.asarray(Wo, dtype=np.float32)
    g_idx = np.arange(E) // 256
    d_idx = np.arange(E) % 64
    bv_exp = bv[g_idx * 64 + d_idx]
    bo_eff = bo + bv_exp @ Wo_f
    out = np.empty((B, S, E), dtype=np.float32)
    for b in range(B):
        yTb = (res.results[2 * b]["yT"].astype(np.float32)
               + res.results[2 * b + 1]["yT"].astype(np.float32))
        out[b] = yTb.T + bo_eff
    if _trace:
        return out, res
    return out
